# revision 9
# baseline (speedup 1.0000x reference)
"""AdditiveAttention (d2l-style) on 8 Trainium2 NeuronCores.

out[b] = softmax_s(mask(w_v . tanh(q[b,l,:] + k[b,s,:]))) @ values[b]
with q = queries @ W_q, k = keys @ W_k, masked to s < valid_lens[b].

Sharding: pure data-parallel over the batch (B=8 -> one batch element per
core); the tiny params are replicated. Per core the dominant cost is the
Lq*Lk*H = 16.7M tanh evaluations on ScalarE.

Device layout (per core):
  qT [h, l] and kT [h, s] live with the hidden dim on partitions, so the
  q+k add is a per-partition tensor_scalar on VectorE and the tanh runs as
  big-free-dim ACT instructions. Scores are built transposed ([s, l]) by
  per-row matvecs (feat chunk as the stationary operand, w_v moving), which
  makes the softmax sum and the attn@values contraction plain PE matmuls
  over the s-partition chunks. A ones-column appended to values yields the
  softmax denominator in the same accumulated matmul; masking folds into the
  exp via a per-partition bias of -50 on masked rows (exp(x-50) ~ 0).
  valid_len == 0 batches replicate the reference's uniform-softmax behavior
  by zeroing w_v and the mask (scores == 0 -> uniform).
"""

import numpy as np
import ml_dtypes

LQ, LK, H = 128, 1024, 128
NCHUNK = LK // 128  # 8 s-chunks of 128
L_B = 16            # query rows per tanh block
NEG_BIAS = -50.0

_BF = ml_dtypes.bfloat16


def _apply_tile_patch():
    """walrus gen3 allows 1 sync-wait per CTRL instruction, but TileContext's
    exit drain carries one wait per outstanding semaphore. Split them into
    single-wait NOPs."""
    import concourse.tile as tile
    from concourse.vector_clock import ScopedClock, VectorClock

    if getattr(tile.TileContext, "_drain_split_patched", False):
        return

    def _patched(self, tick_clock, wait_clock):
        nc = self.nc
        gc = tick_clock.global_clock
        nprocs = len(gc)
        for proc in range(nprocs):
            tick = gc[proc]
            if tick <= 0:
                continue
            mini = VectorClock([0] * nprocs)
            mini.require_at_least(proc, tick)
            nop = nc.sync.nop(nofuse=True, hint="drain_split_wait")
            wait_clock.add_sem_waits(nop.ins, ScopedClock({None: mini}))
        nc.sync.drain()
        nc.all_engine_barrier()
        assert self.sems is not None
        popped = nc._tile_sem_poison_stack.pop()
        assert popped is self._sem_poison
        nc.clear_and_free_semaphores(list(self.sems.allocated().values()))
        nc.all_engine_barrier()

    tile.TileContext._drain_and_barrier = _patched
    tile.TileContext._drain_split_patched = True




def _split_multiwaits(bir_json: bytes) -> bytes:
    """walrus gen3 rejects >1 sync-wait per instruction; hoist extras onto
    single-wait NoOps inserted immediately before (same engine, same block)."""
    import json

    m = json.loads(bir_json)
    n_new = 0
    for func in m["functions"]:
        for bb in func["blocks"]:
            out_insts = []
            changed = False
            for ins in bb["instructions"]:
                sync = ins.get("sync_info") or {}
                waits = sync.get("on_wait") or []
                if len(waits) > 1:
                    changed = True
                    for w in waits[:-1]:
                        n_new += 1
                        out_insts.append({
                            "debug": ins.get("debug"),
                            "engine": ins["engine"],
                            "ins": [],
                            "name": f"{ins['name']}-sw{n_new}",
                            "opcode": "NoOp",
                            "outs": [],
                            "sync_info": {"on_update": [], "on_wait": [w]},
                        })
                    sync["on_wait"] = waits[-1:]
                out_insts.append(ins)
            if changed:
                bb["instructions"] = out_insts
    return json.dumps(m).encode()


def _wrap_to_json_bytes(nc):
    orig = type(nc).to_json_bytes
    nc.to_json_bytes = lambda: _split_multiwaits(orig(nc))
    return nc


def build_nc():
    import concourse.bass as bass
    import concourse.tile as tile
    from concourse import mybir

    _apply_tile_patch()
    bf16 = mybir.dt.bfloat16
    f32 = mybir.dt.float32
    Act = mybir.ActivationFunctionType

    nc = bass.Bass()
    qT_in = nc.declare_dram_parameter("qT", [128, LQ], bf16, isOutput=False)
    kT_in = nc.declare_dram_parameter("kT", [128, LK], bf16, isOutput=False)
    vaug_in = nc.declare_dram_parameter("vaug", [LK, 129], bf16, isOutput=False)
    wq_in = nc.declare_dram_parameter("wq", [128, H], bf16, isOutput=False)
    wk_in = nc.declare_dram_parameter("wk", [128, H], bf16, isOutput=False)
    wv_in = nc.declare_dram_parameter("wv", [H, 1], bf16, isOutput=False)
    mask_in = nc.declare_dram_parameter("mask", [128, NCHUNK], f32, isOutput=False)
    out_ext = nc.declare_dram_parameter("out", [LQ, 128], f32, isOutput=True)

    with tile.TileContext(nc) as tc:
        with tc.tile_pool(name="const", bufs=1) as const, \
             tc.tile_pool(name="feat", bufs=2) as featp, \
             tc.tile_pool(name="psum", bufs=1, space="PSUM") as psum, \
             tc.tile_pool(name="omisc", bufs=1) as omisc:
            wq_sb = const.tile([128, H], bf16)
            nc.sync.dma_start(out=wq_sb[:], in_=wq_in[:])
            wk_sb = const.tile([128, H], bf16)
            nc.sync.dma_start(out=wk_sb[:], in_=wk_in[:])
            wv_sb = const.tile([H, 1], bf16)
            nc.sync.dma_start(out=wv_sb[:], in_=wv_in[:])
            mask_sb = const.tile([128, NCHUNK], f32)
            nc.sync.dma_start(out=mask_sb[:], in_=mask_in[:])
            qTin_sb = const.tile([128, LQ], bf16)
            nc.sync.dma_start(out=qTin_sb[:], in_=qT_in[:])
            kTin_sb = const.tile([128, LK], bf16)
            nc.sync.dma_start(out=kTin_sb[:], in_=kT_in[:])
            vaug_sb = const.tile([128, NCHUNK, 129], bf16)
            nc.sync.dma_start(
                out=vaug_sb[:], in_=vaug_in.rearrange("(c p) n -> p c n", p=128)
            )

            # projections: qT[h,l] = W_q.T @ queries.T, kT[h,s] = W_k.T @ keys.T
            qT_ps = psum.tile([128, LQ], f32)
            nc.tensor.matmul(qT_ps[:], wq_sb[:], qTin_sb[:], start=True, stop=True)
            qT_sb = const.tile([128, LQ], f32)
            nc.vector.tensor_copy(qT_sb[:], qT_ps[:])
            kT_ps = psum.tile([128, LK], f32)
            nc.tensor.matmul(
                kT_ps[:, 0:512], wk_sb[:], kTin_sb[:, 0:512], start=True, stop=True
            )
            nc.tensor.matmul(
                kT_ps[:, 512:1024], wk_sb[:], kTin_sb[:, 512:1024],
                start=True, stop=True,
            )
            kT_sb = const.tile([128, LK], bf16)
            nc.vector.tensor_copy(kT_sb[:], kT_ps[:])

            # scoresT[s, l] per chunk c at columns [c*LQ, (c+1)*LQ)
            scoresT_ps = psum.tile([128, NCHUNK * LQ], f32)
            for lb in range(LQ // L_B):
                feat = featp.tile([128, L_B * LK], bf16)
                for j in range(L_B):
                    l = lb * L_B + j
                    nc.vector.tensor_scalar_add(
                        feat[:, j * LK:(j + 1) * LK], kT_sb[:], qT_sb[:, l:l + 1]
                    )
                nc.scalar.activation(feat[:], feat[:], Act.Tanh)
                # w_v matvec per (l, s-chunk), split into 4 col-groups so the
                # 32-col LDWEIGHTS and N=1 matmuls run concurrently per
                # sub-array; the 4 [32,1] outputs concatenate to the full
                # [128,1] scoresT column.
                for j in range(L_B):
                    l = lb * L_B + j
                    for c in range(NCHUNK):
                        base = (j * NCHUNK + c) * 128
                        for g in range(4):
                            nc.tensor.matmul(
                                scoresT_ps[32 * g:32 * g + 32,
                                           c * LQ + l:c * LQ + l + 1],
                                feat[:, base + 32 * g:base + 32 * g + 32],
                                wv_sb[:],
                                start=True, stop=True, tile_position=(0, 32 * g),
                            )

            # exp with fused mask bias; bf16 output feeds the PE contraction
            expT_sb = omisc.tile([128, NCHUNK * LQ], bf16)
            for c in range(NCHUNK):
                nc.scalar.activation(
                    expT_sb[:, c * LQ:(c + 1) * LQ],
                    scoresT_ps[:, c * LQ:(c + 1) * LQ],
                    Act.Exp,
                    bias=mask_sb[:, c:c + 1],
                    scale=1.0,
                )

            # out'[l, 0:128] = sum_s exp * values ; out'[l, 128] = sum_s exp
            out_ps = psum.tile([128, 129], f32)
            for c in range(NCHUNK):
                nc.tensor.matmul(
                    out_ps[:],
                    expT_sb[:, c * LQ:(c + 1) * LQ],
                    vaug_sb[:, c, :],
                    start=(c == 0), stop=(c == NCHUNK - 1),
                )
            recip = omisc.tile([128, 1], f32)
            nc.vector.reciprocal(recip[:], out_ps[:, 128:129])
            outf = omisc.tile([128, 128], f32)
            nc.vector.tensor_scalar_mul(outf[:], out_ps[:, 0:128], recip[:])
            nc.sync.dma_start(out=out_ext[:], in_=outf[:])
    return _wrap_to_json_bytes(nc)


def _make_in_maps(queries, keys, values, valid_lens, W_q, W_k, w_v):
    queries = np.asarray(queries, dtype=np.float32)
    keys = np.asarray(keys, dtype=np.float32)
    values = np.asarray(values, dtype=np.float32)
    valid_lens = np.asarray(valid_lens)
    W_q = np.asarray(W_q, dtype=np.float32)
    W_k = np.asarray(W_k, dtype=np.float32)
    w_v = np.asarray(w_v, dtype=np.float32)

    B = queries.shape[0]
    wq = np.ascontiguousarray(W_q).astype(_BF)
    wk = np.ascontiguousarray(W_k).astype(_BF)
    ones = np.ones((LK, 1), np.float32)
    in_maps = []
    for b in range(B):
        vl = int(valid_lens[b])
        mask = np.zeros((LK,), np.float32)
        if vl <= 0:
            # reference: softmax over an all-masked row is uniform; scores==0
            # reproduces that exactly.
            wv_b = np.zeros((H, 1), _BF)
        else:
            mask[min(vl, LK):] = NEG_BIAS
            wv_b = np.ascontiguousarray(w_v.reshape(H, 1)).astype(_BF)
        in_maps.append({
            "qT": np.ascontiguousarray(queries[b].T).astype(_BF),
            "kT": np.ascontiguousarray(keys[b].T).astype(_BF),
            "vaug": np.concatenate([values[b], ones], axis=1).astype(_BF),
            "wq": wq,
            "wk": wk,
            "wv": wv_b,
            "mask": np.ascontiguousarray(mask.reshape(NCHUNK, 128).T.astype(np.float32)),
        })
    return in_maps


_NC_CACHE = [None]


def _run(in_maps, trace=False, tmpdir=None):
    from concourse.bass_utils import run_bass_kernel_spmd

    if _NC_CACHE[0] is None:
        _NC_CACHE[0] = build_nc()
    nc = _NC_CACHE[0]
    return run_bass_kernel_spmd(
        nc, in_maps, core_ids=list(range(8)), trace=trace, tmpdir=tmpdir
    )


def kernel(queries, keys, values, valid_lens, W_q, W_k, w_v):
    in_maps = _make_in_maps(queries, keys, values, valid_lens, W_q, W_k, w_v)
    res = _run(in_maps, trace=False)
    return np.stack(
        [np.asarray(res.results[i]["out"], dtype=np.float32) for i in range(len(in_maps))],
        axis=0,
    )


def kernel_traced(queries, keys, values, valid_lens, W_q, W_k, w_v, tmpdir=None):
    """Like kernel() but profiles the run; returns (out, exec_time_ns)."""
    in_maps = _make_in_maps(queries, keys, values, valid_lens, W_q, W_k, w_v)
    res = _run(in_maps, trace=True, tmpdir=tmpdir)
    out = np.stack(
        [np.asarray(res.results[i]["out"], dtype=np.float32) for i in range(len(in_maps))],
        axis=0,
    )
    return out, res.exec_time_ns
